# revision 20
# baseline (speedup 1.0000x reference)
"""Multi-head attention (B=2, S=2048, D=1024, H=16) on 8 trn2 NeuronCores.

Sharding: data-parallel over batch (cores 0-3 -> b=0, cores 4-7 -> b=1),
tensor-parallel over heads (4 heads per core, with the matching column/row
shards of Wq/Wk/Wv/Wo).

Per-core kernel (Bass/Tile):
  - QKV projections from a host-pre-transposed fp16 x^T with an appended ones
    row, so q/k/v biases ride along as an extra contraction row (K=1025).
    q-side is pre-scaled by 1/sqrt(HD) on the host.
  - Scores are computed twice on the PE (natural orientation [qi,kj] for the
    softmax + attn output; transposed [kj,qi] for the P@V matmul, whose
    contraction dim must live on partitions).  exp() runs on the scalar
    engine; row sums come for free via accum_out.
  - P@V accumulates ctx^T [64, 2048] per head in PSUM; normalization happens
    once on the small ctx^T (recip row broadcast across partitions via
    doubling DMAs), the big attn tensor is normalized by a per-partition
    tensor_scalar multiply.
  - Output projection contracts the local 256 ctx dims against Wo's matching
    column shard (host-transposed), with bo/4 folded in via a ones row; the
    four cores of a batch produce partial sums that the host adds.

Matmul inputs are fp16 (full PE streaming rate); all accumulation is fp32.
"""

import numpy as np

B, S, D, H = 2, 2048, 1024, 16
HD = D // H  # 64
NCORES = 8
HEADS_PER_CORE = H // 4  # 4 heads per core
HLOC = HEADS_PER_CORE * HD  # 256 local head dims
KAUG = D + 1  # contraction with ones row
KCH = 8  # full 128-row K chunks (plus one 1-row chunk)

_CACHE = {}


def _broadcast_rows(nc, dst, src_row, nrows):
    """Copy src_row [1, N] into dst[0:nrows, :] by doubling SBUF->SBUF DMAs."""
    nc.sync.dma_start(dst[0:1, :], src_row[0:1, :])
    filled = 1
    while filled < nrows:
        n = min(filled, nrows - filled)
        nc.sync.dma_start(dst[filled : filled + n, :], dst[0:n, :])
        filled += n


def _build_kernel(ctx, tc, xT, wqT, wkT, wvT, woT, attn_d, out_d):
    import concourse.mybir as mybir

    nc = tc.nc
    f32 = mybir.dt.float32
    f16 = mybir.dt.float16
    EXP = mybir.ActivationFunctionType.Exp

    from contextlib import ExitStack

    qkv_pool = ctx.enter_context(tc.tile_pool(name="qkv", bufs=1))
    small_pool = ctx.enter_context(tc.tile_pool(name="small", bufs=1))

    # qT/kT: [j_local, s] with j on partitions (2 tiles of [128, S])
    qT = [qkv_pool.tile([128, S], f16, tag=f"qT{i}", name=f"qT{i}") for i in range(2)]
    kT = [qkv_pool.tile([128, S], f16, tag=f"kT{i}", name=f"kT{i}") for i in range(2)]
    v_sb = [
        qkv_pool.tile([128, HLOC], f16, tag=f"v{st}", name=f"v{st}") for st in range(16)
    ]
    ctxT = [
        qkv_pool.tile([128, S], f16, tag=f"ctxT{i}", name=f"ctxT{i}") for i in range(2)
    ]

    with ExitStack() as pctx:
        const_pool = pctx.enter_context(tc.tile_pool(name="consts", bufs=1))
        ppsum = pctx.enter_context(tc.tile_pool(name="ppsum", bufs=2, space="PSUM"))

        def load_chunks(src, ncols, label, ones_last):
            # fp16 chunks straight from DRAM; the final row is the ones row
            # for x (synthesized on chip) or the DMA'd bias row for weights.
            chunks = []
            for kc in range(KCH):
                t = const_pool.tile(
                    [128, ncols], f16, tag=f"{label}{kc}", name=f"{label}{kc}"
                )
                nc.sync.dma_start(t[:], src[kc * 128 : (kc + 1) * 128, :])
                chunks.append(t)
            t8 = const_pool.tile([1, ncols], f16, tag=f"{label}8", name=f"{label}8")
            if ones_last:
                nc.vector.memset(t8[:], 1.0)
            else:
                nc.sync.dma_start(t8[:], src[D : D + 1, :])
            chunks.append(t8)
            return chunks

        xt = load_chunks(xT, S, "xt", True)
        wq = load_chunks(wqT, HLOC, "wq", False)
        wk = load_chunks(wkT, HLOC, "wk", False)
        wv = load_chunks(wvT, HLOC, "wv", False)

        # ---- projections ----
        for w_ch, dst in ((wq, qT), (wk, kT)):
            for mt in range(2):
                for nt in range(4):
                    ps = ppsum.tile([128, 512], f32, tag="proj_ps", name="proj_ps")
                    for kc in range(KCH + 1):
                        nc.tensor.matmul(
                            ps[:],
                            lhsT=w_ch[kc][:, mt * 128 : (mt + 1) * 128],
                            rhs=xt[kc][:, nt * 512 : (nt + 1) * 512],
                            start=(kc == 0),
                            stop=(kc == KCH),
                        )
                    nc.vector.tensor_copy(dst[mt][:, nt * 512 : (nt + 1) * 512], ps[:])

        # v: natural [s, j_local] (16 tiles of [128, HLOC]) for P@V stationary side
        for st in range(16):
            ps = ppsum.tile([128, HLOC], f32, tag="projv_ps", name="projv_ps")
            for kc in range(KCH + 1):
                nc.tensor.matmul(
                    ps[:],
                    lhsT=xt[kc][:, st * 128 : (st + 1) * 128],
                    rhs=wv[kc][:],
                    start=(kc == 0),
                    stop=(kc == KCH),
                )
            nc.vector.tensor_copy(v_sb[st][:], ps[:])

    # ---- attention, one head at a time ----
    work_pool = ctx.enter_context(tc.tile_pool(name="work", bufs=2))
    actx = ctx.enter_context(ExitStack())
    apsum = actx.enter_context(tc.tile_pool(name="apsum", bufs=2, space="PSUM"))
    for h in range(HEADS_PER_CORE):
        tidx, row0 = divmod(h, 2)
        row0 *= HD
        qh = qT[tidx][row0 : row0 + HD, :]  # [64, S]
        kh = kT[tidx][row0 : row0 + HD, :]

        # -- natural orientation: scores -> exp(+rowsum) -> normalize -> DMA out
        rsig = small_pool.tile([128, 16], f32, tag=f"rsig{h}", name=f"rsig{h}")
        for qt in range(16):
            sig_half = []
            attn_t = work_pool.tile([128, S], f32, tag="attn_t", name="attn_t")
            for hf in range(2):
                ps = apsum.tile([128, 1024], f32, tag="s_ps", name="sn_ps")
                for nt in range(2):
                    col = hf * 1024 + nt * 512
                    nc.tensor.matmul(
                        ps[:, nt * 512 : (nt + 1) * 512],
                        lhsT=qh[:, qt * 128 : (qt + 1) * 128],
                        rhs=kh[:, col : col + 512],
                        start=True,
                        stop=True,
                    )
                e_nat = work_pool.tile([128, 1024], f32, tag="e_nat", name="e_nat")
                sg = small_pool.tile(
                    [128, 1], f32, tag=f"sg{hf}", name=f"sg{hf}", bufs=2
                )
                nc.scalar.activation(e_nat[:], ps[:], EXP, accum_out=sg[:])
                sig_half.append((e_nat, sg))
            sig = small_pool.tile([128, 1], f32, tag="sig", name="sig", bufs=2)
            nc.vector.tensor_add(sig[:], sig_half[0][1][:], sig_half[1][1][:])
            nc.vector.reciprocal(rsig[:, qt : qt + 1], sig[:])
            for hf in range(2):
                nc.vector.tensor_scalar_mul(
                    attn_t[:, hf * 1024 : (hf + 1) * 1024],
                    sig_half[hf][0][:],
                    rsig[:, qt : qt + 1],
                )
            nc.sync.dma_start(
                attn_d[h * S + qt * 128 : h * S + (qt + 1) * 128, :], attn_t[:]
            )

        # -- transposed orientation: scores^T -> exp -> P@V accumulation
        ps_ctx = apsum.tile([64, S], f32, tag="ctx_ps", name="ctx_ps", bufs=1)
        for kc in range(16):
            for hf in range(2):
                ps = apsum.tile([128, 1024], f32, tag="s_ps", name="st_ps")
                for nt in range(2):
                    col = hf * 1024 + nt * 512
                    nc.tensor.matmul(
                        ps[:, nt * 512 : (nt + 1) * 512],
                        lhsT=kh[:, kc * 128 : (kc + 1) * 128],
                        rhs=qh[:, col : col + 512],
                        start=True,
                        stop=True,
                    )
                eT = work_pool.tile([128, 1024], f16, tag="eT", name="eT")
                nc.scalar.activation(eT[:], ps[:], EXP)
                for nt in range(2):
                    col = hf * 1024 + nt * 512
                    nc.tensor.matmul(
                        ps_ctx[:, col : col + 512],
                        lhsT=v_sb[kc][:, h * HD : (h + 1) * HD],
                        rhs=eT[:, nt * 512 : (nt + 1) * 512],
                        start=(kc == 0),
                        stop=(kc == 15),
                    )

        # -- normalize ctx^T rows by recip(rowsum) broadcast across partitions
        rrow = small_pool.tile([1, S], f32, tag="rrow", name="rrow")
        for t in range(16):
            nc.sync.dma_start(rrow[0:1, t * 128 : (t + 1) * 128], rsig[:, t : t + 1])
        bc = small_pool.tile([64, S], f32, tag="bc", name="bc")
        _broadcast_rows(nc, bc, rrow, 64)
        nc.vector.tensor_mul(ctxT[tidx][row0 : row0 + HD, :], ps_ctx[:], bc[:])

    actx.close()

    # ---- output projection: out_part[s, o] = ctx_local @ WoT_local + bo/4 ----
    wo_pool = ctx.enter_context(tc.tile_pool(name="wo_pool", bufs=1))
    opsum = ctx.enter_context(tc.tile_pool(name="opsum", bufs=2, space="PSUM"))
    wo = []
    for cc in range(3):
        rows = 128 if cc < 2 else 1
        t = wo_pool.tile([rows, D], f16, tag=f"wo{cc}", name=f"wo{cc}")
        nc.sync.dma_start(t[:], woT[cc * 128 : cc * 128 + rows, :])
        wo.append(t)
    wob = wo[2]
    ones_row = wo_pool.tile([1, S], f16, tag="ones_row", name="ones_row")
    nc.vector.memset(ones_row[:], 1.0)

    for st in range(16):
        ps = opsum.tile([128, 1024], f32, tag="o_ps", name="o_ps")
        for nt in range(2):
            osl = slice(nt * 512, (nt + 1) * 512)
            for cc in range(2):
                nc.tensor.matmul(
                    ps[:, osl],
                    lhsT=ctxT[cc][:, st * 128 : (st + 1) * 128],
                    rhs=wo[cc][:, osl],
                    start=(cc == 0),
                    stop=False,
                )
            nc.tensor.matmul(
                ps[:, osl],
                lhsT=ones_row[:, st * 128 : (st + 1) * 128],
                rhs=wob[:, osl],
                start=False,
                stop=True,
            )
        out_sb = work_pool.tile([128, D], f32, tag="out_sb", name="out_sb")
        nc.vector.tensor_copy(out_sb[:], ps[:])
        nc.sync.dma_start(out_d[st * 128 : (st + 1) * 128, :], out_sb[:])


def _get_bass():
    if "nc" in _CACHE:
        return _CACHE["nc"]
    from contextlib import ExitStack

    import concourse.mybir as mybir
    import concourse.tile as tile
    from concourse import bacc

    f32 = mybir.dt.float32
    f16 = mybir.dt.float16
    nc = bacc.Bacc("TRN2", target_bir_lowering=False, debug=False, num_devices=NCORES)
    xT = nc.dram_tensor("xT", [KAUG, S], f16, kind="ExternalInput").ap()
    wqT = nc.dram_tensor("wqT", [KAUG, HLOC], f16, kind="ExternalInput").ap()
    wkT = nc.dram_tensor("wkT", [KAUG, HLOC], f16, kind="ExternalInput").ap()
    wvT = nc.dram_tensor("wvT", [KAUG, HLOC], f16, kind="ExternalInput").ap()
    woT = nc.dram_tensor("woT", [HLOC + 1, D], f16, kind="ExternalInput").ap()
    attn_d = nc.dram_tensor(
        "attn_part", [HEADS_PER_CORE * S, S], f32, kind="ExternalOutput"
    ).ap()
    out_d = nc.dram_tensor("out_part", [S, D], f32, kind="ExternalOutput").ap()

    with tile.TileContext(nc) as tc:
        with ExitStack() as ctx:
            _build_kernel(ctx, tc, xT, wqT, wkT, wvT, woT, attn_d, out_d)

    nc.compile()
    _CACHE["nc"] = nc
    return nc


def _in_maps(x, Wq, bq, Wk, bk, Wv, bv, Wo, bo):
    x = np.asarray(x, np.float32)
    scale = 1.0 / np.sqrt(np.float32(HD))
    maps = []
    ones = np.ones((1, S), np.float32)
    for c in range(NCORES):
        b = c // 4
        sl = slice((c % 4) * HLOC, (c % 4 + 1) * HLOC)
        xTa = np.concatenate([x[b].T, ones], axis=0)
        wqTa = np.concatenate([Wq[sl, :].T * scale, bq[None, sl] * scale], axis=0)
        wkTa = np.concatenate([Wk[sl, :].T, bk[None, sl]], axis=0)
        wvTa = np.concatenate([Wv[sl, :].T, bv[None, sl]], axis=0)
        woTa = np.concatenate([Wo[:, sl].T, bo[None, :] / 4.0], axis=0)
        maps.append(
            {
                "xT": np.ascontiguousarray(xTa, dtype=np.float16),
                "wqT": np.ascontiguousarray(wqTa, dtype=np.float16),
                "wkT": np.ascontiguousarray(wkTa, dtype=np.float16),
                "wvT": np.ascontiguousarray(wvTa, dtype=np.float16),
                "woT": np.ascontiguousarray(woTa, dtype=np.float16),
            }
        )
    return maps


def run(inputs, trace=False):
    """Run on hardware; returns ((out, attn), BassKernelResults)."""
    from concourse.bass_utils import run_bass_kernel_spmd

    nc = _get_bass()
    maps = _in_maps(**inputs)
    res = run_bass_kernel_spmd(nc, maps, core_ids=list(range(NCORES)), trace=trace)
    attn = np.stack(
        [r["attn_part"].reshape(HEADS_PER_CORE, S, S) for r in res.results]
    ).reshape(B, H, S, S)
    out = np.stack(
        [
            res.results[0]["out_part"]
            + res.results[1]["out_part"]
            + res.results[2]["out_part"]
            + res.results[3]["out_part"],
            res.results[4]["out_part"]
            + res.results[5]["out_part"]
            + res.results[6]["out_part"]
            + res.results[7]["out_part"],
        ]
    )
    return (out, attn), res


def kernel(x, Wq, bq, Wk, bk, Wv, bv, Wo, bo):
    (out, attn), _ = run(
        dict(x=x, Wq=Wq, bq=bq, Wk=Wk, bk=bk, Wv=Wv, bv=bv, Wo=Wo, bo=bo)
    )
    return out, attn


# revision 21
# speedup vs baseline: 31474.0552x; 31474.0552x over previous
"""Multi-head attention (B=2, S=2048, D=1024, H=16) on 8 trn2 NeuronCores.

Sharding: data-parallel over batch (cores 0-3 -> b=0, cores 4-7 -> b=1),
tensor-parallel over heads (4 heads per core, with the matching column/row
shards of Wq/Wk/Wv/Wo).

Per-core kernel (Bass/Tile):
  - QKV projections from a host-pre-transposed fp16 x^T with an appended ones
    row, so q/k/v biases ride along as an extra contraction row (K=1025).
    q-side is pre-scaled by 1/sqrt(HD) on the host.
  - Scores are computed twice on the PE (natural orientation [qi,kj] for the
    softmax + attn output; transposed [kj,qi] for the P@V matmul, whose
    contraction dim must live on partitions).  exp() runs on the scalar
    engine; row sums come for free via accum_out.
  - P@V accumulates ctx^T [64, 2048] per head in PSUM; normalization happens
    once on the small ctx^T (recip row broadcast across partitions via
    doubling DMAs), the big attn tensor is normalized by a per-partition
    tensor_scalar multiply.
  - Output projection contracts the local 256 ctx dims against Wo's matching
    column shard (host-transposed), with bo/4 folded in via a ones row; the
    four cores of a batch produce partial sums that the host adds.

Matmul inputs are fp16 (full PE streaming rate); all accumulation is fp32.
"""

import numpy as np

B, S, D, H = 2, 2048, 1024, 16
HD = D // H  # 64
NCORES = 8
HEADS_PER_CORE = H // 4  # 4 heads per core
HLOC = HEADS_PER_CORE * HD  # 256 local head dims
KAUG = D + 1  # contraction with ones row
KCH = 8  # full 128-row K chunks (plus one 1-row chunk)

_CACHE = {}


def _broadcast_rows(nc, dst, src_row, nrows):
    """Copy src_row [1, N] into dst[0:nrows, :] by doubling SBUF->SBUF DMAs."""
    nc.sync.dma_start(dst[0:1, :], src_row[0:1, :])
    filled = 1
    while filled < nrows:
        n = min(filled, nrows - filled)
        nc.sync.dma_start(dst[filled : filled + n, :], dst[0:n, :])
        filled += n


def _build_kernel(ctx, tc, xT, wqT, wkT, wvT, woT, attn_d, out_d):
    import concourse.mybir as mybir

    nc = tc.nc
    f32 = mybir.dt.float32
    f16 = mybir.dt.float16
    EXP = mybir.ActivationFunctionType.Exp

    from contextlib import ExitStack

    qkv_pool = ctx.enter_context(tc.tile_pool(name="qkv", bufs=1))
    small_pool = ctx.enter_context(tc.tile_pool(name="small", bufs=1))

    # qT/kT: [j_local, s] with j on partitions (2 tiles of [128, S])
    qT = [qkv_pool.tile([128, S], f16, tag=f"qT{i}", name=f"qT{i}") for i in range(2)]
    kT = [qkv_pool.tile([128, S], f16, tag=f"kT{i}", name=f"kT{i}") for i in range(2)]
    v_sb = [
        qkv_pool.tile([128, HLOC], f16, tag=f"v{st}", name=f"v{st}") for st in range(16)
    ]
    ctxT = [
        qkv_pool.tile([128, S], f16, tag=f"ctxT{i}", name=f"ctxT{i}") for i in range(2)
    ]

    with ExitStack() as pctx:
        const_pool = pctx.enter_context(tc.tile_pool(name="consts", bufs=1))
        ppsum = pctx.enter_context(tc.tile_pool(name="ppsum", bufs=2, space="PSUM"))

        def load_chunks(src, ncols, label, ones_last):
            # fp16 chunks straight from DRAM; the final row is the ones row
            # for x (synthesized on chip) or the DMA'd bias row for weights.
            chunks = []
            for kc in range(KCH):
                t = const_pool.tile(
                    [128, ncols], f16, tag=f"{label}{kc}", name=f"{label}{kc}"
                )
                nc.sync.dma_start(t[:], src[kc * 128 : (kc + 1) * 128, :])
                chunks.append(t)
            t8 = const_pool.tile([1, ncols], f16, tag=f"{label}8", name=f"{label}8")
            if ones_last:
                nc.vector.memset(t8[:], 1.0)
            else:
                nc.sync.dma_start(t8[:], src[D : D + 1, :])
            chunks.append(t8)
            return chunks

        xt = load_chunks(xT, S, "xt", True)
        wq = load_chunks(wqT, HLOC, "wq", False)
        wk = load_chunks(wkT, HLOC, "wk", False)
        wv = load_chunks(wvT, HLOC, "wv", False)

        # ---- projections ----
        for w_ch, dst in ((wq, qT), (wk, kT)):
            for mt in range(2):
                for nt in range(4):
                    ps = ppsum.tile([128, 512], f32, tag="proj_ps", name="proj_ps")
                    for kc in range(KCH + 1):
                        nc.tensor.matmul(
                            ps[:],
                            lhsT=w_ch[kc][:, mt * 128 : (mt + 1) * 128],
                            rhs=xt[kc][:, nt * 512 : (nt + 1) * 512],
                            start=(kc == 0),
                            stop=(kc == KCH),
                        )
                    nc.vector.tensor_copy(dst[mt][:, nt * 512 : (nt + 1) * 512], ps[:])

        # v: natural [s, j_local] (16 tiles of [128, HLOC]) for P@V stationary side
        for st in range(16):
            ps = ppsum.tile([128, HLOC], f32, tag="projv_ps", name="projv_ps")
            for kc in range(KCH + 1):
                nc.tensor.matmul(
                    ps[:],
                    lhsT=xt[kc][:, st * 128 : (st + 1) * 128],
                    rhs=wv[kc][:],
                    start=(kc == 0),
                    stop=(kc == KCH),
                )
            nc.vector.tensor_copy(v_sb[st][:], ps[:])

    # ---- attention, one head at a time ----
    work_pool = ctx.enter_context(tc.tile_pool(name="work", bufs=2))
    actx = ctx.enter_context(ExitStack())
    apsum = actx.enter_context(tc.tile_pool(name="apsum", bufs=2, space="PSUM"))
    for h in range(HEADS_PER_CORE):
        tidx, row0 = divmod(h, 2)
        row0 *= HD
        qh = qT[tidx][row0 : row0 + HD, :]  # [64, S]
        kh = kT[tidx][row0 : row0 + HD, :]

        # -- natural orientation: scores -> exp(+rowsum) -> normalize -> DMA out
        rsig = small_pool.tile([128, 16], f32, tag=f"rsig{h}", name=f"rsig{h}")
        for qt in range(16):
            sig_half = []
            attn_t = work_pool.tile([128, S], f16, tag="attn_t", name="attn_t")
            for hf in range(2):
                ps = apsum.tile([128, 1024], f32, tag="s_ps", name="sn_ps")
                for nt in range(2):
                    col = hf * 1024 + nt * 512
                    nc.tensor.matmul(
                        ps[:, nt * 512 : (nt + 1) * 512],
                        lhsT=qh[:, qt * 128 : (qt + 1) * 128],
                        rhs=kh[:, col : col + 512],
                        start=True,
                        stop=True,
                    )
                e_nat = work_pool.tile([128, 1024], f32, tag="e_nat", name="e_nat", bufs=3)
                sg = small_pool.tile(
                    [128, 1], f32, tag=f"sg{hf}", name=f"sg{hf}", bufs=2
                )
                nc.scalar.activation(e_nat[:], ps[:], EXP, accum_out=sg[:])
                sig_half.append((e_nat, sg))
            sig = small_pool.tile([128, 1], f32, tag="sig", name="sig", bufs=2)
            nc.vector.tensor_add(sig[:], sig_half[0][1][:], sig_half[1][1][:])
            nc.vector.reciprocal(rsig[:, qt : qt + 1], sig[:])
            for hf in range(2):
                nc.vector.tensor_scalar_mul(
                    attn_t[:, hf * 1024 : (hf + 1) * 1024],
                    sig_half[hf][0][:],
                    rsig[:, qt : qt + 1],
                )
            nc.sync.dma_start(
                attn_d[h * S + qt * 128 : h * S + (qt + 1) * 128, :], attn_t[:]
            )

        # -- transposed orientation: scores^T -> exp -> P@V accumulation
        ps_ctx = apsum.tile([64, S], f32, tag="ctx_ps", name="ctx_ps", bufs=1)
        for kc in range(16):
            for hf in range(2):
                ps = apsum.tile([128, 1024], f32, tag="s_ps", name="st_ps")
                for nt in range(2):
                    col = hf * 1024 + nt * 512
                    nc.tensor.matmul(
                        ps[:, nt * 512 : (nt + 1) * 512],
                        lhsT=kh[:, kc * 128 : (kc + 1) * 128],
                        rhs=qh[:, col : col + 512],
                        start=True,
                        stop=True,
                    )
                eT = work_pool.tile([128, 1024], f16, tag="eT", name="eT", bufs=3)
                nc.scalar.activation(eT[:], ps[:], EXP)
                for nt in range(2):
                    col = hf * 1024 + nt * 512
                    nc.tensor.matmul(
                        ps_ctx[:, col : col + 512],
                        lhsT=v_sb[kc][:, h * HD : (h + 1) * HD],
                        rhs=eT[:, nt * 512 : (nt + 1) * 512],
                        start=(kc == 0),
                        stop=(kc == 15),
                    )

        # -- normalize ctx^T rows by recip(rowsum) broadcast across partitions
        rrow = small_pool.tile([1, S], f32, tag="rrow", name="rrow", bufs=2)
        for t in range(16):
            nc.sync.dma_start(rrow[0:1, t * 128 : (t + 1) * 128], rsig[:, t : t + 1])
        bc = small_pool.tile([64, S], f32, tag="bc", name="bc", bufs=2)
        _broadcast_rows(nc, bc, rrow, 64)
        nc.vector.tensor_mul(ctxT[tidx][row0 : row0 + HD, :], ps_ctx[:], bc[:])

    actx.close()

    # ---- output projection: out_part[s, o] = ctx_local @ WoT_local + bo/4 ----
    wo_pool = ctx.enter_context(tc.tile_pool(name="wo_pool", bufs=1))
    opsum = ctx.enter_context(tc.tile_pool(name="opsum", bufs=2, space="PSUM"))
    wo = []
    for cc in range(3):
        rows = 128 if cc < 2 else 1
        t = wo_pool.tile([rows, D], f16, tag=f"wo{cc}", name=f"wo{cc}")
        nc.sync.dma_start(t[:], woT[cc * 128 : cc * 128 + rows, :])
        wo.append(t)
    wob = wo[2]
    ones_row = wo_pool.tile([1, S], f16, tag="ones_row", name="ones_row")
    nc.vector.memset(ones_row[:], 1.0)

    for st in range(16):
        ps = opsum.tile([128, 1024], f32, tag="o_ps", name="o_ps")
        for nt in range(2):
            osl = slice(nt * 512, (nt + 1) * 512)
            for cc in range(2):
                nc.tensor.matmul(
                    ps[:, osl],
                    lhsT=ctxT[cc][:, st * 128 : (st + 1) * 128],
                    rhs=wo[cc][:, osl],
                    start=(cc == 0),
                    stop=False,
                )
            nc.tensor.matmul(
                ps[:, osl],
                lhsT=ones_row[:, st * 128 : (st + 1) * 128],
                rhs=wob[:, osl],
                start=False,
                stop=True,
            )
        out_sb = work_pool.tile([128, D], f32, tag="out_sb", name="out_sb")
        nc.vector.tensor_copy(out_sb[:], ps[:])
        nc.sync.dma_start(out_d[st * 128 : (st + 1) * 128, :], out_sb[:])


def _get_bass():
    if "nc" in _CACHE:
        return _CACHE["nc"]
    from contextlib import ExitStack

    import concourse.mybir as mybir
    import concourse.tile as tile
    from concourse import bacc

    f32 = mybir.dt.float32
    f16 = mybir.dt.float16
    nc = bacc.Bacc("TRN2", target_bir_lowering=False, debug=False, num_devices=NCORES)
    xT = nc.dram_tensor("xT", [KAUG, S], f16, kind="ExternalInput").ap()
    wqT = nc.dram_tensor("wqT", [KAUG, HLOC], f16, kind="ExternalInput").ap()
    wkT = nc.dram_tensor("wkT", [KAUG, HLOC], f16, kind="ExternalInput").ap()
    wvT = nc.dram_tensor("wvT", [KAUG, HLOC], f16, kind="ExternalInput").ap()
    woT = nc.dram_tensor("woT", [HLOC + 1, D], f16, kind="ExternalInput").ap()
    attn_d = nc.dram_tensor(
        "attn_part", [HEADS_PER_CORE * S, S], f16, kind="ExternalOutput"
    ).ap()
    out_d = nc.dram_tensor("out_part", [S, D], f32, kind="ExternalOutput").ap()

    with tile.TileContext(nc) as tc:
        with ExitStack() as ctx:
            _build_kernel(ctx, tc, xT, wqT, wkT, wvT, woT, attn_d, out_d)

    nc.compile()
    _CACHE["nc"] = nc
    return nc


def _in_maps(x, Wq, bq, Wk, bk, Wv, bv, Wo, bo):
    x = np.asarray(x, np.float32)
    scale = 1.0 / np.sqrt(np.float32(HD))
    maps = []
    ones = np.ones((1, S), np.float32)
    for c in range(NCORES):
        b = c // 4
        sl = slice((c % 4) * HLOC, (c % 4 + 1) * HLOC)
        xTa = np.concatenate([x[b].T, ones], axis=0)
        wqTa = np.concatenate([Wq[sl, :].T * scale, bq[None, sl] * scale], axis=0)
        wkTa = np.concatenate([Wk[sl, :].T, bk[None, sl]], axis=0)
        wvTa = np.concatenate([Wv[sl, :].T, bv[None, sl]], axis=0)
        woTa = np.concatenate([Wo[:, sl].T, bo[None, :] / 4.0], axis=0)
        maps.append(
            {
                "xT": np.ascontiguousarray(xTa, dtype=np.float16),
                "wqT": np.ascontiguousarray(wqTa, dtype=np.float16),
                "wkT": np.ascontiguousarray(wkTa, dtype=np.float16),
                "wvT": np.ascontiguousarray(wvTa, dtype=np.float16),
                "woT": np.ascontiguousarray(woTa, dtype=np.float16),
            }
        )
    return maps


def run(inputs, trace=False):
    """Run on hardware; returns ((out, attn), BassKernelResults)."""
    from concourse.bass_utils import run_bass_kernel_spmd

    nc = _get_bass()
    maps = _in_maps(**inputs)
    res = run_bass_kernel_spmd(nc, maps, core_ids=list(range(NCORES)), trace=trace)
    attn = np.stack(
        [r["attn_part"].reshape(HEADS_PER_CORE, S, S) for r in res.results]
    ).reshape(B, H, S, S).astype(np.float32)
    out = np.stack(
        [
            res.results[0]["out_part"]
            + res.results[1]["out_part"]
            + res.results[2]["out_part"]
            + res.results[3]["out_part"],
            res.results[4]["out_part"]
            + res.results[5]["out_part"]
            + res.results[6]["out_part"]
            + res.results[7]["out_part"],
        ]
    )
    return (out, attn), res


def kernel(x, Wq, bq, Wk, bk, Wv, bv, Wo, bo):
    (out, attn), _ = run(
        dict(x=x, Wq=Wq, bq=bq, Wk=Wk, bk=bk, Wv=Wv, bv=bv, Wo=Wo, bo=bo)
    )
    return out, attn
